# revision 15
# baseline (speedup 1.0000x reference)
"""CRF log-likelihood kernel for Trainium2 (Bass/Tile), 8-core data parallel.

Math (per sequence b):
  out[b] = gold_path_score(b) - logZ(b)

logZ is computed on-device with an exp-domain forward DP over T steps:
  u_{t} = el_t ⊙ (W^T u_{t-1})
where u is kept in linear (exp) space, W is a block-diagonal augmented
transition operator built from exp(trans), and el_t = exp(logit_t - CSHIFT).

Label-space augmentation per group: 32 real labels + 1 "sink" label.
- W[i, sink] = 1 (i active), W[sink, sink] = 1, W[sink, j<32] = 0.
- el[sink, t] = (t >= seq_len): at t == seq_len the sink captures exactly
  sum_i u_{len-1}[i] = Z * exp(-scales); afterwards it self-loops untouched.
- host scatters logits[b, seq_len(b), :] = -1e9 (when len < T) so the active
  labels die exactly at t == len; an extra step t == T freezes len == T rows.
Three extra "ones" output rows give per-group column sums每 step for free;
every RENORM_EVERY steps columns are rescaled by 1/colsum (exactly tracked
via ACT-Ln of the applied factor, telescoped into the final answer).

Layout per core (128 sequences):
  partitions 0..95  = active labels, p = 32*g + j   (3 groups)
  partitions 96..98 = sink row of group g
  psum rows 99..101 = per-group column sums (matmul ones-rows)
  columns 0..42     = sequences of the group: b_local = 43*g + c
Host does: index gathers for the gold-path score (labels/trans/seq_lens only),
layout shuffling, and the final [B]-sized subtraction.
"""

import numpy as np

B, T, L = 1024, 512, 32
NCORES = 8
BPC = B // NCORES        # 128 sequences per core
G = 3                    # label groups per core
NCOL = 43                # columns per group (group 2 uses 42 + 1 pad)
NACT = 96                # active label partitions
NPART = 99               # + 3 sink rows
MOUT = 102               # + 3 colsum rows
CSHIFT = 4.5
TEX = T + 1              # time slices 0..T (incl. the extra freeze step)
TCH = 57                 # el build chunk (9 * 57 = 513)
NCH = TEX // TCH
RENORM_EVERY = 128       # renorm at t = 128, 256, 384

_prog_cache = {}
last_result = None       # BassKernelResults of the most recent run (for test.py)


def _build_program():
    import concourse.bacc as bacc
    import concourse.tile as tile
    from concourse import mybir

    f32 = mybir.dt.float32
    AF = mybir.ActivationFunctionType

    nc = bacc.Bacc("TRN2", target_bir_lowering=False, debug=False, num_devices=NCORES)
    lg = nc.dram_tensor("lg", [NACT, TEX, NCOL], f32, kind="ExternalInput")
    el32 = nc.dram_tensor("el32", [G, TEX, NCOL], f32, kind="ExternalInput")
    w = nc.dram_tensor("w", [NPART, MOUT], f32, kind="ExternalInput")
    wb = nc.dram_tensor("wb", [2 * G, NPART], f32, kind="ExternalInput")
    res = nc.dram_tensor("res", [G, NCOL], f32, kind="ExternalOutput")

    with tile.TileContext(nc) as tc:
        with (
            tc.tile_pool(name="big", bufs=1) as big,
            tc.tile_pool(name="stage", bufs=3) as stage_p,
            tc.tile_pool(name="consts", bufs=1) as consts,
            tc.tile_pool(name="u", bufs=3) as upool,
            tc.tile_pool(name="small", bufs=4) as small,
            tc.tile_pool(name="fin", bufs=1) as fin,
            tc.tile_pool(name="ps", bufs=4, space="PSUM") as pspool,
            tc.tile_pool(name="psr", bufs=2, space="PSUM") as psrpool,
        ):
            el_sb = big.tile([NPART, TEX, NCOL], f32)
            w_sb = consts.tile([NPART, MOUT], f32)
            wb_sb = consts.tile([2 * G, NPART], f32)
            biasc = consts.tile([128, 1], f32)
            nc.vector.memset(biasc[:], -CSHIFT)

            nc.sync.dma_start(out=w_sb[:], in_=w[:])
            nc.sync.dma_start(out=wb_sb[:], in_=wb[:])
            # sink rows: el32 values land on partitions 96..98 (one aligned DMA)
            nc.sync.dma_start(out=el_sb[NACT:NPART, :, :], in_=el32[:])
            # active rows: stage raw logits, then bulk exp into el_sb
            for ch in range(NCH):
                st = stage_p.tile([NACT, TCH, NCOL], f32, tag="stage")
                t0 = ch * TCH
                nc.sync.dma_start(out=st[:], in_=lg[:, t0 : t0 + TCH, :])
                nc.scalar.activation(
                    el_sb[0:NACT, t0 : t0 + TCH, :], st[:], AF.Exp, bias=biasc[0:NACT, :]
                )

            lnrs = []
            uprev = el_sb[:, 0, :]
            for t in range(1, T + 1):
                ps = pspool.tile([MOUT, NCOL], f32, tag="ps")
                nc.tensor.matmul(ps[:], w_sb[:], uprev, start=True, stop=True)
                un = upool.tile([NPART, NCOL], f32, tag="u")
                nc.vector.tensor_mul(un[:], ps[0:NPART, :], el_sb[:, t, :])
                if t % RENORM_EVERY == 0 and t < T:
                    e = len(lnrs)
                    # [96:102] = sink rows + colsum rows (32-aligned partition start);
                    # +eps guards 1/0 on the (unused) sink-reciprocal rows
                    ts6 = small.tile([2 * G, NCOL], f32, tag="ts6")
                    nc.vector.tensor_scalar_add(ts6[:], ps[NACT : NACT + 2 * G, :], 1e-30)
                    rr6 = small.tile([2 * G, NCOL], f32, tag="rr6")
                    nc.vector.reciprocal(rr6[:], ts6[:])
                    psr = psrpool.tile([NPART, NCOL], f32, tag="psr")
                    nc.tensor.matmul(psr[:], wb_sb[:], rr6[:], start=True, stop=True)
                    un2 = upool.tile([NPART, NCOL], f32, tag="u2")
                    nc.vector.tensor_mul(un2[:], psr[:], un[:])
                    # psr sink rows hold exactly the applied per-group factor
                    lnr = fin.tile([G, NCOL], f32, tag=f"lnr{e}")
                    nc.scalar.activation(lnr[:], psr[NACT:NPART, :], AF.Ln)
                    lnrs.append(lnr)
                    uprev = un2[:]
                else:
                    uprev = un[:]

            # res = ln(u_sink) - sum_e ln(r_e)
            acc = fin.tile([G, NCOL], f32, tag="lnu")
            nc.scalar.activation(acc[:], uprev[NACT:NPART, :], AF.Ln)
            for e, lnr in enumerate(lnrs):
                nxt = fin.tile([G, NCOL], f32, tag=f"facc{e}")
                nc.vector.tensor_sub(nxt[:], acc[:], lnr[:])
                acc = nxt
            nc.sync.dma_start(out=res[:], in_=acc[:])

    nc.compile()
    return nc


def _host_prep(logits, trans, labels, seq_lens):
    logits = np.ascontiguousarray(np.asarray(logits), dtype=np.float32)
    trans = np.asarray(trans, dtype=np.float32)
    labels = np.asarray(labels)
    lens = np.clip(np.asarray(seq_lens), 1, T).astype(np.int64)

    # ---- gold path score (host: index gathers over small inputs) ----
    tmask = np.arange(T)[None, :] < lens[:, None]
    unary = np.take_along_axis(logits, labels[..., None].astype(np.int64), axis=2)[..., 0]
    gp = (unary * tmask).sum(1) + (trans[labels[:, :-1], labels[:, 1:]] * tmask[:, 1:]).sum(1)

    # ---- device-input construction ----
    lgx = logits.copy()
    scat = lens < T
    lgx[np.where(scat)[0], lens[scat], :] = -1e9  # kill active labels at t == len
    lgx = np.concatenate([lgx, np.zeros((B, 1, L), np.float32)], axis=1)  # t == T slice

    el32 = (np.arange(TEX)[None, :] >= lens[:, None]).astype(np.float32)  # [B, 513]

    # per-core [G, 32, TEX, NCOL] and [G, TEX, NCOL]; pad column = a dummy
    # sequence with len == T (finite everywhere, result discarded)
    lg_cores, el32_cores = [], []
    for core in range(NCORES):
        b0 = core * BPC
        lgp = np.zeros((G, 32, TEX, NCOL), np.float32)
        e32 = np.zeros((G, TEX, NCOL), np.float32)
        for g in range(G):
            ncols = NCOL if g < 2 else BPC - 2 * NCOL
            bs = b0 + g * NCOL
            # [ncols, TEX, 32] -> [32, TEX, ncols]
            blk = lgx[bs : bs + ncols].transpose(2, 1, 0)
            lgp[g, :, :, :ncols] = blk
            e32[g, :, :ncols] = el32[bs : bs + ncols].T
            if ncols < NCOL:  # pad column: dummy len == T sequence
                e32[g, T, ncols:] = 1.0
        lg_cores.append(np.ascontiguousarray(lgp).reshape(NACT, TEX, NCOL))
        el32_cores.append(e32)

    # ---- augmented transition operator ----
    E = np.exp(trans).astype(np.float32)  # E[i, j]: score i -> j
    W = np.zeros((NPART, MOUT), np.float32)
    wb = np.zeros((2 * G, NPART), np.float32)
    for g in range(G):
        W[32 * g : 32 * (g + 1), 32 * g : 32 * (g + 1)] = E
        W[32 * g : 32 * (g + 1), NACT + g] = 1.0   # active -> sink collect
        W[NACT + g, NACT + g] = 1.0                # sink self-loop
        W[32 * g : 32 * (g + 1), NPART + g] = 1.0  # colsum row
        W[NACT + g, NPART + g] = 1.0
        # broadcast lhsT: row G+g of the reciprocal tile = 1/colsum of group g
        wb[G + g, 32 * g : 32 * (g + 1)] = 1.0
        wb[G + g, NACT + g] = 1.0

    return gp, lens, lg_cores, el32_cores, W, wb


def _log(msg):
    import time as _t

    print(f"[kernel {_t.strftime('%H:%M:%S')}] {msg}", flush=True)


def kernel(logits, trans, labels, seq_lens):
    global last_result
    from concourse.bass_utils import run_bass_kernel_spmd

    _log("host prep start")
    gp, lens, lg_cores, el32_cores, W, wb = _host_prep(logits, trans, labels, seq_lens)
    _log("host prep done")

    if "nc" not in _prog_cache:
        _prog_cache["nc"] = _build_program()
        _log("program built")
    nc = _prog_cache["nc"]

    in_maps = [
        {"lg": lg_cores[i], "el32": el32_cores[i], "w": W, "wb": wb}
        for i in range(NCORES)
    ]
    r = run_bass_kernel_spmd(nc, in_maps, core_ids=list(range(NCORES)))
    last_result = r
    _log("device run done")

    # ---- unshard: res[g, c] -> logZ[b] ----
    dev = np.zeros(B, np.float32)
    for core in range(NCORES):
        rc = r.results[core]["res"]  # [G, NCOL]
        b0 = core * BPC
        for g in range(G):
            ncols = NCOL if g < 2 else BPC - 2 * NCOL
            dev[b0 + g * NCOL : b0 + g * NCOL + ncols] = rc[g, :ncols]

    logZ = dev + CSHIFT * lens.astype(np.float32)
    return (gp - logZ).astype(np.float32)


# revision 17
# speedup vs baseline: 1.5719x; 1.5719x over previous
"""CRF log-likelihood kernel for Trainium2 (Bass/Tile), 8-core data parallel.

Math (per sequence b):
  out[b] = gold_path_score(b) - logZ(b)

logZ is computed on-device with an exp-domain forward DP over T steps:
  u_{t} = el_t ⊙ (W^T u_{t-1})
where u is kept in linear (exp) space, W is a block-diagonal augmented
transition operator built from exp(trans), and el_t = exp(logit_t - CSHIFT).

Label-space augmentation per group: 32 real labels + 1 "sink" label.
- W[i, sink] = 1 (i active), W[sink, sink] = 1, W[sink, j<32] = 0.
- el[sink, t] = (t >= seq_len): at t == seq_len the sink captures exactly
  sum_i u_{len-1}[i] = Z * exp(-scales); afterwards it self-loops untouched.
- host scatters logits[b, seq_len(b), :] = -1e9 (when len < T) so the active
  labels die exactly at t == len; an extra step t == T freezes len == T rows.
Three extra "ones" output rows give per-group column sums每 step for free;
every RENORM_EVERY steps columns are rescaled by 1/colsum (exactly tracked
via ACT-Ln of the applied factor, telescoped into the final answer).

Layout per core (128 sequences):
  partitions 0..95  = active labels, p = 32*g + j   (3 groups)
  partitions 96..98 = sink row of group g
  psum rows 99..101 = per-group column sums (matmul ones-rows)
  columns 0..42     = sequences of the group: b_local = 43*g + c
Host does: index gathers for the gold-path score (labels/trans/seq_lens only),
layout shuffling, and the final [B]-sized subtraction.
"""

import numpy as np
import ml_dtypes

B, T, L = 1024, 512, 32
NCORES = 8
BPC = B // NCORES        # 128 sequences per core
G = 3                    # label groups per core
NCOL = 43                # columns per group (group 2 uses 42 + 1 pad)
NACT = 96                # active label partitions
NPART = 99               # + 3 sink rows
MOUT = 102               # + 3 colsum rows
CSHIFT = 4.5
TEX = T + 1              # time slices 0..T (incl. the extra freeze step)
TCH = 57                 # el build chunk (9 * 57 = 513)
NCH = TEX // TCH
RENORM_EVERY = 128       # renorm at t = 128, 256, 384

_prog_cache = {}
last_result = None       # BassKernelResults of the most recent run (for test.py)


def _build_program():
    import concourse.bacc as bacc
    import concourse.tile as tile
    from concourse import mybir

    f32 = mybir.dt.float32
    bf16 = mybir.dt.bfloat16
    AF = mybir.ActivationFunctionType

    nc = bacc.Bacc("TRN2", target_bir_lowering=False, debug=False, num_devices=NCORES)
    lg = nc.dram_tensor("lg", [NACT, TEX, NCOL], f32, kind="ExternalInput")
    el32 = nc.dram_tensor("el32", [G, TEX, NCOL], bf16, kind="ExternalInput")
    w = nc.dram_tensor("w", [NPART, MOUT], bf16, kind="ExternalInput")
    wb = nc.dram_tensor("wb", [2 * G, NPART], f32, kind="ExternalInput")
    res = nc.dram_tensor("res", [G, NCOL], f32, kind="ExternalOutput")

    with tile.TileContext(nc) as tc:
        with (
            tc.tile_pool(name="big", bufs=1) as big,
            tc.tile_pool(name="stage", bufs=3) as stage_p,
            tc.tile_pool(name="consts", bufs=1) as consts,
            tc.tile_pool(name="u", bufs=3) as upool,
            tc.tile_pool(name="small", bufs=4) as small,
            tc.tile_pool(name="fin", bufs=1) as fin,
            tc.tile_pool(name="ps", bufs=4, space="PSUM") as pspool,
            tc.tile_pool(name="psr", bufs=2, space="PSUM") as psrpool,
        ):
            el_sb = big.tile([NPART, TEX, NCOL], bf16)
            w_sb = consts.tile([NPART, MOUT], bf16)
            wb_sb = consts.tile([2 * G, NPART], f32)
            biasc = consts.tile([128, 1], f32)
            nc.vector.memset(biasc[:], -CSHIFT)

            nc.sync.dma_start(out=w_sb[:], in_=w[:])
            nc.sync.dma_start(out=wb_sb[:], in_=wb[:])
            # sink rows: el32 values land on partitions 96..98 (one aligned DMA)
            nc.sync.dma_start(out=el_sb[NACT:NPART, :, :], in_=el32[:])
            # active rows: stage raw logits, then bulk exp into el_sb
            for ch in range(NCH):
                st = stage_p.tile([NACT, TCH, NCOL], f32, tag="stage")
                t0 = ch * TCH
                nc.sync.dma_start(out=st[:], in_=lg[:, t0 : t0 + TCH, :])
                nc.scalar.activation(
                    el_sb[0:NACT, t0 : t0 + TCH, :], st[:], AF.Exp, bias=biasc[0:NACT, :]
                )

            lnrs = []
            uprev = el_sb[:, 0, :]
            for t in range(1, T + 1):
                ps = pspool.tile([MOUT, NCOL], f32, tag="ps")
                nc.tensor.matmul(ps[:], w_sb[:], uprev, start=True, stop=True)
                un = upool.tile([NPART, NCOL], bf16, tag="u")
                nc.vector.tensor_mul(un[:], ps[0:NPART, :], el_sb[:, t, :])
                if t % RENORM_EVERY == 0 and t < T:
                    e = len(lnrs)
                    # [96:102] = sink rows + colsum rows (32-aligned partition start);
                    # +eps guards 1/0 on the (unused) sink-reciprocal rows
                    ts6 = small.tile([2 * G, NCOL], f32, tag="ts6")
                    nc.vector.tensor_scalar_add(ts6[:], ps[NACT : NACT + 2 * G, :], 1e-30)
                    rr6 = small.tile([2 * G, NCOL], f32, tag="rr6")
                    nc.vector.reciprocal(rr6[:], ts6[:])
                    psr = psrpool.tile([NPART, NCOL], f32, tag="psr")
                    nc.tensor.matmul(psr[:], wb_sb[:], rr6[:], start=True, stop=True)
                    un2 = upool.tile([NPART, NCOL], bf16, tag="u2")
                    nc.vector.tensor_mul(un2[:], psr[:], un[:])
                    # psr sink rows hold exactly the applied per-group factor
                    lnr = fin.tile([G, NCOL], f32, tag=f"lnr{e}")
                    nc.scalar.activation(lnr[:], psr[NACT:NPART, :], AF.Ln)
                    lnrs.append(lnr)
                    uprev = un2[:]
                else:
                    uprev = un[:]

            # res = ln(u_sink) - sum_e ln(r_e)
            acc = fin.tile([G, NCOL], f32, tag="lnu")
            nc.scalar.activation(acc[:], uprev[NACT:NPART, :], AF.Ln)
            for e, lnr in enumerate(lnrs):
                nxt = fin.tile([G, NCOL], f32, tag=f"facc{e}")
                nc.vector.tensor_sub(nxt[:], acc[:], lnr[:])
                acc = nxt
            nc.sync.dma_start(out=res[:], in_=acc[:])

    nc.compile()
    return nc


def _host_prep(logits, trans, labels, seq_lens):
    logits = np.ascontiguousarray(np.asarray(logits), dtype=np.float32)
    trans = np.asarray(trans, dtype=np.float32)
    labels = np.asarray(labels)
    lens = np.clip(np.asarray(seq_lens), 1, T).astype(np.int64)

    # ---- gold path score (host: index gathers over small inputs) ----
    tmask = np.arange(T)[None, :] < lens[:, None]
    unary = np.take_along_axis(logits, labels[..., None].astype(np.int64), axis=2)[..., 0]
    gp = (unary * tmask).sum(1) + (trans[labels[:, :-1], labels[:, 1:]] * tmask[:, 1:]).sum(1)

    # ---- device-input construction ----
    lgx = logits.copy()
    scat = lens < T
    lgx[np.where(scat)[0], lens[scat], :] = -1e9  # kill active labels at t == len
    lgx = np.concatenate([lgx, np.zeros((B, 1, L), np.float32)], axis=1)  # t == T slice

    el32 = (np.arange(TEX)[None, :] >= lens[:, None]).astype(np.float32)  # [B, 513]

    # per-core [G, 32, TEX, NCOL] and [G, TEX, NCOL]; pad column = a dummy
    # sequence with len == T (finite everywhere, result discarded)
    lg_cores, el32_cores = [], []
    for core in range(NCORES):
        b0 = core * BPC
        lgp = np.zeros((G, 32, TEX, NCOL), np.float32)
        e32 = np.zeros((G, TEX, NCOL), np.float32)
        for g in range(G):
            ncols = NCOL if g < 2 else BPC - 2 * NCOL
            bs = b0 + g * NCOL
            # [ncols, TEX, 32] -> [32, TEX, ncols]
            blk = lgx[bs : bs + ncols].transpose(2, 1, 0)
            lgp[g, :, :, :ncols] = blk
            e32[g, :, :ncols] = el32[bs : bs + ncols].T
            if ncols < NCOL:  # pad column: dummy len == T sequence
                e32[g, T, ncols:] = 1.0
        lg_cores.append(np.ascontiguousarray(lgp).reshape(NACT, TEX, NCOL))
        el32_cores.append(e32.astype(ml_dtypes.bfloat16))

    # ---- augmented transition operator ----
    E = np.exp(trans).astype(np.float32)  # E[i, j]: score i -> j
    W = np.zeros((NPART, MOUT), np.float32)
    wb = np.zeros((2 * G, NPART), np.float32)
    for g in range(G):
        W[32 * g : 32 * (g + 1), 32 * g : 32 * (g + 1)] = E
        W[32 * g : 32 * (g + 1), NACT + g] = 1.0   # active -> sink collect
        W[NACT + g, NACT + g] = 1.0                # sink self-loop
        W[32 * g : 32 * (g + 1), NPART + g] = 1.0  # colsum row
        W[NACT + g, NPART + g] = 1.0
        # broadcast lhsT: row G+g of the reciprocal tile = 1/colsum of group g
        wb[G + g, 32 * g : 32 * (g + 1)] = 1.0
        wb[G + g, NACT + g] = 1.0

    return gp, lens, lg_cores, el32_cores, W.astype(ml_dtypes.bfloat16), wb


def _log(msg):
    import time as _t

    print(f"[kernel {_t.strftime('%H:%M:%S')}] {msg}", flush=True)


def kernel(logits, trans, labels, seq_lens):
    global last_result
    from concourse.bass_utils import run_bass_kernel_spmd

    _log("host prep start")
    gp, lens, lg_cores, el32_cores, W, wb = _host_prep(logits, trans, labels, seq_lens)
    _log("host prep done")

    if "nc" not in _prog_cache:
        _prog_cache["nc"] = _build_program()
        _log("program built")
    nc = _prog_cache["nc"]

    in_maps = [
        {"lg": lg_cores[i], "el32": el32_cores[i], "w": W, "wb": wb}
        for i in range(NCORES)
    ]
    r = run_bass_kernel_spmd(nc, in_maps, core_ids=list(range(NCORES)))
    last_result = r
    _log("device run done")

    # ---- unshard: res[g, c] -> logZ[b] ----
    dev = np.zeros(B, np.float32)
    for core in range(NCORES):
        rc = r.results[core]["res"]  # [G, NCOL]
        b0 = core * BPC
        for g in range(G):
            ncols = NCOL if g < 2 else BPC - 2 * NCOL
            dev[b0 + g * NCOL : b0 + g * NCOL + ncols] = rc[g, :ncols]

    logZ = dev + CSHIFT * lens.astype(np.float32)
    return (gp - logZ).astype(np.float32)


# revision 21
# speedup vs baseline: 2.8075x; 1.7860x over previous
"""CRF log-likelihood kernel for Trainium2 (Bass/Tile), 8-core data parallel.

out[b] = gold_path_score(b) - logZ(b)

logZ via exp-domain DP with forward and backward chains meeting at t = F:
  fwd:  u_t   = el_t  ⊙ (Wf^T u_{t-1}),      t = 1..F      (u_0 = el_0)
  bwd:  γ_σ   = Wb^T (el_{T+1-σ} ⊙ γ_{σ-1}), σ = 1..T-F    (γ_0 = sink)
Sequences with len <= F finish inside the fwd chain via an absorbing "sink"
label that captures sum_i u_{len-1}[i] exactly at t == len; longer sequences
use the midpoint identity Z = Σ_j α_F[j]·β_F[j], with the bwd chain's sink
"birthing" β = 1 at each sequence's own end time. The two chains are
independent, so PE matmuls of one overlap DVE multiplies of the other.

Layout per core (128 sequences):
  partitions 0..95 = active labels (3 groups x 32), 96..98 = sink row per
  group; psum rows 99..101 = per-group column sums (ones-columns of the
  stationary operand). columns: b_local = 43*g + c.
Scaling: all emissions carry e^{-CSHIFT}; columns are renormalized by their
column sum mid-chain (factor tracked exactly via ACT-Ln of the applied
multiplier). Host adds CSHIFT*len back and picks sink vs combine per length.
Host also does the gold-path gathers (labels/trans only) and final subtract.
"""

import numpy as np
import ml_dtypes

B, T, L = 1024, 512, 32
NCORES = 8
BPC = B // NCORES        # 128 sequences per core
G = 3                    # label groups per core
NCOL = 43                # columns per group (group 2 uses 42 + 1 pad)
NACT = 96                # active label partitions
NPART = 99               # + 3 sink rows
MOUT = 102               # + 3 colsum rows
CSHIFT = 4.5
TEX = T + 1              # el time slices 0..T
TCH = 57                 # el build chunk (9 * 57 = 513)
NCH = TEX // TCH
F = 256                  # fwd ticks; bwd ticks = T - F
SB = T - F
RENORM_EVERY = 128

_prog_cache = {}
last_result = None       # BassKernelResults of the most recent run (for test.py)


def _build_program():
    import concourse.bacc as bacc
    import concourse.tile as tile
    from concourse import mybir

    f32 = mybir.dt.float32
    bf16 = mybir.dt.bfloat16
    AF = mybir.ActivationFunctionType

    nc = bacc.Bacc("TRN2", target_bir_lowering=False, debug=False, num_devices=NCORES)
    lg = nc.dram_tensor("lg", [NACT, TEX, NCOL], f32, kind="ExternalInput")
    el32 = nc.dram_tensor("el32", [G, TEX, NCOL], bf16, kind="ExternalInput")
    wf = nc.dram_tensor("wf", [NPART, MOUT], bf16, kind="ExternalInput")
    wbk = nc.dram_tensor("wbk", [NPART, MOUT], bf16, kind="ExternalInput")
    wbc = nc.dram_tensor("wbc", [2 * G, NPART], f32, kind="ExternalInput")
    wcs = nc.dram_tensor("wcs", [NPART, G], bf16, kind="ExternalInput")
    resf = nc.dram_tensor("resf", [G, NCOL], f32, kind="ExternalOutput")
    resc = nc.dram_tensor("resc", [G, NCOL], f32, kind="ExternalOutput")

    with tile.TileContext(nc) as tc:
        with (
            tc.tile_pool(name="big", bufs=1) as big,
            tc.tile_pool(name="stage", bufs=3) as stage_p,
            tc.tile_pool(name="consts", bufs=1) as consts,
            tc.tile_pool(name="u", bufs=3) as upool,
            tc.tile_pool(name="v", bufs=3) as vpool,
            tc.tile_pool(name="small", bufs=4) as small,
            tc.tile_pool(name="fin", bufs=1) as fin,
            tc.tile_pool(name="psf", bufs=3, space="PSUM") as psfpool,
            tc.tile_pool(name="psb", bufs=3, space="PSUM") as psbpool,
            tc.tile_pool(name="psx", bufs=2, space="PSUM") as psxpool,
        ):
            el_sb = big.tile([NPART, TEX, NCOL], bf16)
            wf_sb = consts.tile([NPART, MOUT], bf16)
            wb_sb = consts.tile([NPART, MOUT], bf16)
            wbc_sb = consts.tile([2 * G, NPART], f32)
            wcs_sb = consts.tile([NPART, G], bf16)
            biasc = consts.tile([128, 1], f32)
            g0 = consts.tile([NPART, NCOL], bf16)
            nc.vector.memset(biasc[:], -CSHIFT)
            nc.vector.memset(g0[:], 0.0)
            nc.vector.memset(g0[NACT:NPART, :], 1.0)

            nc.sync.dma_start(out=wf_sb[:], in_=wf[:])
            nc.sync.dma_start(out=wb_sb[:], in_=wbk[:])
            nc.sync.dma_start(out=wbc_sb[:], in_=wbc[:])
            nc.sync.dma_start(out=wcs_sb[:], in_=wcs[:])
            # sink rows land on partitions 96..98 (one aligned DMA)
            nc.sync.dma_start(out=el_sb[NACT:NPART, :, :], in_=el32[:])
            # active rows: stage raw logits, bulk-exp into el_sb.
            # build order alternates ends: bwd consumes slices from t=T down.
            order = []
            lo, hi = 0, NCH - 1
            while lo <= hi:
                order.append(hi)
                if lo != hi:
                    order.append(lo)
                hi -= 1
                lo += 1
            for ch in order:
                st = stage_p.tile([NACT, TCH, NCOL], f32, tag="stage")
                t0 = ch * TCH
                nc.sync.dma_start(out=st[:], in_=lg[:, t0 : t0 + TCH, :])
                nc.scalar.activation(
                    el_sb[0:NACT, t0 : t0 + TCH, :], st[:], AF.Exp, bias=biasc[0:NACT, :]
                )

            lnrs_f, lnrs_b = [], []
            uprev = el_sb[:, 0, :]
            gprev = g0[:]
            gprev_sbuf = True
            ulast = None
            pb_last = None
            pend_renorm = None
            for k in range(1, max(F, SB) + 1):
                # ---- fwd tick t = k ----
                if k <= F:
                    psf = psfpool.tile([MOUT, NCOL], f32, tag="psf")
                    nc.tensor.matmul(psf[:], wf_sb[:], uprev, start=True, stop=True)
                    un = upool.tile([NPART, NCOL], bf16, tag="u")
                    nc.vector.tensor_mul(un[:], psf[0:NPART, :], el_sb[:, k, :])
                    if k % RENORM_EVERY == 0 and k < F:
                        ts6 = small.tile([2 * G, NCOL], f32, tag="ts6f")
                        nc.vector.tensor_scalar_add(
                            ts6[:], psf[NACT : NACT + 2 * G, :], 1e-30
                        )
                        rr6 = small.tile([2 * G, NCOL], f32, tag="rr6f")
                        nc.vector.reciprocal(rr6[:], ts6[:])
                        psr = psxpool.tile([NPART, NCOL], f32, tag="psr")
                        nc.tensor.matmul(psr[:], wbc_sb[:], rr6[:], start=True, stop=True)
                        un2 = upool.tile([NPART, NCOL], bf16, tag="u2")
                        nc.vector.tensor_mul(un2[:], psr[:], un[:])
                        lnr = fin.tile([G, NCOL], f32, tag=f"lnrf{len(lnrs_f)}")
                        nc.scalar.activation(lnr[:], psr[NACT:NPART, :], AF.Ln)
                        lnrs_f.append(lnr)
                        uprev = un2[:]
                    else:
                        uprev = un[:]
                    if k == F:
                        ulast = uprev
                # ---- bwd tick σ = k, el time T+1-k ----
                if k <= SB:
                    vn = vpool.tile([NPART, NCOL], bf16, tag="v")
                    src = gprev if gprev_sbuf else gprev[0:NPART, :]
                    nc.vector.tensor_mul(vn[:], src, el_sb[:, T + 1 - k, :])
                    if pend_renorm is not None:
                        # apply the deferred renorm factor (can't read two
                        # PSUM operands in one TT)
                        vn2 = vpool.tile([NPART, NCOL], bf16, tag="v2")
                        nc.vector.tensor_mul(vn2[:], pend_renorm[:], vn[:])
                        vn = vn2
                        pend_renorm = None
                    gprev_sbuf = False
                    psb = psbpool.tile([MOUT, NCOL], f32, tag="psb")
                    nc.tensor.matmul(psb[:], wb_sb[:], vn[:], start=True, stop=True)
                    if k % RENORM_EVERY == 0 and k < SB:
                        ts6b = small.tile([2 * G, NCOL], f32, tag="ts6b")
                        nc.vector.tensor_scalar_add(
                            ts6b[:], psb[NACT : NACT + 2 * G, :], 1e-30
                        )
                        rr6b = small.tile([2 * G, NCOL], f32, tag="rr6b")
                        nc.vector.reciprocal(rr6b[:], ts6b[:])
                        psrb = psxpool.tile([NPART, NCOL], f32, tag="psr")
                        nc.tensor.matmul(
                            psrb[:], wbc_sb[:], rr6b[:], start=True, stop=True
                        )
                        pend_renorm = psrb
                        lnrb = fin.tile([G, NCOL], f32, tag=f"lnrb{len(lnrs_b)}")
                        nc.scalar.activation(lnrb[:], psrb[NACT:NPART, :], AF.Ln)
                        lnrs_b.append(lnrb)
                    gprev = psb
                    if k == SB:
                        pb_last = (gprev, gprev_sbuf)

            # ---- combine: w = u_F ⊙ γ_S; Zc = per-group colsum of w ----
            gl, gl_sbuf = pb_last
            wt = vpool.tile([NPART, NCOL], bf16, tag="wt")
            nc.vector.tensor_mul(wt[:], gl if gl_sbuf else gl[0:NPART, :], ulast)
            psc = psxpool.tile([G, NCOL], f32, tag="psr")
            nc.tensor.matmul(psc[:], wcs_sb[:], wt[:], start=True, stop=True)

            # resf = ln(u_F sink) - Σ lnr_f ; resc = ln(Zc) - Σ lnr_f - Σ lnr_b
            accf = fin.tile([G, NCOL], f32, tag="lnu")
            nc.scalar.activation(accf[:], ulast[NACT:NPART, :], AF.Ln)
            for e, lnr in enumerate(lnrs_f):
                nx = fin.tile([G, NCOL], f32, tag=f"fa{e}")
                nc.vector.tensor_sub(nx[:], accf[:], lnr[:])
                accf = nx
            nc.sync.dma_start(out=resf[:], in_=accf[:])

            accc = fin.tile([G, NCOL], f32, tag="lnc")
            nc.scalar.activation(accc[:], psc[:], AF.Ln)
            for e, lnr in enumerate(lnrs_f + lnrs_b):
                nx = fin.tile([G, NCOL], f32, tag=f"ca{e}")
                nc.vector.tensor_sub(nx[:], accc[:], lnr[:])
                accc = nx
            nc.sync.dma_start(out=resc[:], in_=accc[:])

    nc.compile()
    return nc


def _host_prep(logits, trans, labels, seq_lens):
    logits = np.ascontiguousarray(np.asarray(logits), dtype=np.float32)
    trans = np.asarray(trans, dtype=np.float32)
    labels = np.asarray(labels)
    lens = np.clip(np.asarray(seq_lens), 1, T).astype(np.int64)

    # ---- gold path score (host: index gathers over small inputs) ----
    tmask = np.arange(T)[None, :] < lens[:, None]
    unary = np.take_along_axis(logits, labels[..., None].astype(np.int64), axis=2)[..., 0]
    gp = (unary * tmask).sum(1) + (trans[labels[:, :-1], labels[:, 1:]] * tmask[:, 1:]).sum(1)

    # ---- device inputs: mask every t >= len; pad slice t=T = -inf ----
    lgx = logits.copy()
    lgx[~tmask] = -1e9
    lgx = np.concatenate([lgx, np.full((B, 1, L), -1e9, np.float32)], axis=1)

    el32 = (np.arange(TEX)[None, :] >= lens[:, None]).astype(np.float32)  # [B, 513]

    lg_cores, el32_cores = [], []
    for core in range(NCORES):
        b0 = core * BPC
        lgp = np.full((G, 32, TEX, NCOL), -1e9, np.float32)
        e32 = np.zeros((G, TEX, NCOL), np.float32)
        for g in range(G):
            ncols = NCOL if g < 2 else BPC - 2 * NCOL
            bs = b0 + g * NCOL
            lgp[g, :, :, :ncols] = lgx[bs : bs + ncols].transpose(2, 1, 0)
            e32[g, :, :ncols] = el32[bs : bs + ncols].T
            if ncols < NCOL:  # pad column: dummy len==T sequence, active el = 0
                e32[g, T, ncols:] = 1.0
        lg_cores.append(np.ascontiguousarray(lgp).reshape(NACT, TEX, NCOL))
        el32_cores.append(e32.astype(ml_dtypes.bfloat16))

    # ---- stationary operators ----
    E = np.exp(trans).astype(np.float32)
    Wf = np.zeros((NPART, MOUT), np.float32)
    Wb = np.zeros((NPART, MOUT), np.float32)
    Wbc = np.zeros((2 * G, NPART), np.float32)
    Wcs = np.zeros((NPART, G), np.float32)
    for g in range(G):
        a, sk, cs = 32 * g, NACT + g, NPART + g
        Wf[a : a + 32, a : a + 32] = E
        Wf[a : a + 32, sk] = 1.0
        Wf[sk, sk] = 1.0
        Wf[a : a + 32, cs] = 1.0
        Wf[sk, cs] = 1.0
        Wb[a : a + 32, a : a + 32] = E.T
        Wb[sk, a : a + 32] = 1.0   # sink births β = 1 over all labels
        Wb[sk, sk] = 1.0
        Wb[a : a + 32, cs] = 1.0
        Wb[sk, cs] = 1.0
        Wbc[G + g, a : a + 32] = 1.0
        Wbc[G + g, sk] = 1.0
        Wcs[a : a + 32, g] = 1.0
        Wcs[sk, g] = 1.0
    bf = ml_dtypes.bfloat16
    return gp, lens, lg_cores, el32_cores, Wf.astype(bf), Wb.astype(bf), Wbc, Wcs.astype(bf)


def _log(msg):
    import time as _t

    print(f"[kernel {_t.strftime('%H:%M:%S')}] {msg}", flush=True)


def kernel(logits, trans, labels, seq_lens):
    global last_result
    from concourse.bass_utils import run_bass_kernel_spmd

    _log("host prep start")
    gp, lens, lg_cores, el32_cores, Wf, Wb, Wbc, Wcs = _host_prep(
        logits, trans, labels, seq_lens
    )
    _log("host prep done")

    if "nc" not in _prog_cache:
        _prog_cache["nc"] = _build_program()
        _log("program built")
    nc = _prog_cache["nc"]

    in_maps = [
        {
            "lg": lg_cores[i],
            "el32": el32_cores[i],
            "wf": Wf,
            "wbk": Wb,
            "wbc": Wbc,
            "wcs": Wcs,
        }
        for i in range(NCORES)
    ]
    r = run_bass_kernel_spmd(nc, in_maps, core_ids=list(range(NCORES)))
    last_result = r
    _log("device run done")

    # ---- unshard + select sink vs combine per sequence length ----
    devf = np.zeros(B, np.float32)
    devc = np.zeros(B, np.float32)
    for core in range(NCORES):
        rf = r.results[core]["resf"]
        rc = r.results[core]["resc"]
        b0 = core * BPC
        for g in range(G):
            ncols = NCOL if g < 2 else BPC - 2 * NCOL
            devf[b0 + g * NCOL : b0 + g * NCOL + ncols] = rf[g, :ncols]
            devc[b0 + g * NCOL : b0 + g * NCOL + ncols] = rc[g, :ncols]

    dev = np.where(lens <= F, devf, devc)
    logZ = dev + CSHIFT * lens.astype(np.float32)
    return (gp - logZ).astype(np.float32)
